# revision 10
# baseline (speedup 1.0000x reference)
"""Additive (Bahdanau) attention kernel for one TRN2 chip (8 NeuronCores).

Computes, for query (B,D), keys (B,S,D), mask (B,S), W1 (A,D), W2 (A,D), v (A,):
    scores[b,s] = v . tanh(W1 @ query[b] + W2 @ keys[b,s])
    out = softmax(scores - 1e30 * ~mask, axis=-1)

Sharding: data-parallel over batch B across the 8 cores (4 batches/core);
W1/W2/v replicated. No collectives needed; per-core outputs are concatenated
on the host.

Per-core device kernel (main matmuls in float32r at full PE rate):
  - w1q[a,b]    = W1 @ q_b              (tiny matmul, a on partitions)
  - per (s-tile of 512, b), per a-block j of 128:
        psum[a,s] += W2T_blk.T @ keysT_tile    (8 k-blocks, PE)
        comb = tanh(psum + w1q[:,b])           (ScalarE, per-partition bias)
        acc  += v_j * comb                     (VectorE mul+add chain)
    last add writes acc in f32r; a one-hot ones matmul per (s-tile, b)
    partition-reduces acc into row b of a shared [4, 512] psum tile
  - scores[:, s-tile] = sc_psum + maskadd      (additive -1e30 mask)
  - softmax tail on [4, 2048]: -max -> exp(+bias, accum sum) -> recip -> scale

DMA order is staged (q, W1/W2 a-block j=0, first keys tile, then remaining
a-blocks per j) so the PE starts ~8us in instead of waiting for 10 MB.
"""

import numpy as np

B, S, D, A = 32, 2048, 1024, 1024
NCORES = 8
BL = B // NCORES  # 4 batches per core
ST = 512          # s-tile width
NST = S // ST     # 4 s-tiles per batch
KB = D // 128     # 8 contraction blocks
JB = A // 128     # 8 attn-dim blocks
MASK_NEG = 1e30

_cache = {}


def _build_nc():
    from contextlib import ExitStack

    import concourse.tile as tile
    from concourse import bacc, mybir

    f32 = mybir.dt.float32
    f32r = mybir.dt.float32r
    Tanh = mybir.ActivationFunctionType.Tanh
    Exp = mybir.ActivationFunctionType.Exp

    nc = bacc.Bacc(
        "TRN2",
        target_bir_lowering=False,
        debug=False,
        enable_asserts=False,
        num_devices=NCORES,
    )

    keysT = nc.dram_tensor("keysT", [D, BL, S], f32r, kind="ExternalInput").ap()
    w2t = nc.dram_tensor("w2t", [128, KB, A], f32r, kind="ExternalInput").ap()
    w1t = nc.dram_tensor("w1t", [128, KB, A], f32r, kind="ExternalInput").ap()
    qT = nc.dram_tensor("qT", [128, KB, BL], f32r, kind="ExternalInput").ap()
    # vcol[p, j] = v[j*128+p] — per-partition scalar for the DVE multiply
    vcol = nc.dram_tensor("vcol", [128, JB], f32, kind="ExternalInput").ap()
    # onesz[p, b, c] = 1 if b == c else 0 — one-hot ones column per batch so
    # each batch's partition-reduce lands in its own psum row
    onesz = nc.dram_tensor("onesz", [128, BL * BL], f32r, kind="ExternalInput").ap()
    maskadd = nc.dram_tensor("maskadd", [BL, S], f32, kind="ExternalInput").ap()
    out = nc.dram_tensor("out", [BL, S], f32, kind="ExternalOutput").ap()

    keysT_r = keysT.rearrange("(k p) b s -> p k b s", p=128)

    with tile.TileContext(nc) as tc, ExitStack() as ctx:
        singles = ctx.enter_context(tc.tile_pool(name="singles", bufs=1))
        keysp = ctx.enter_context(tc.tile_pool(name="keys", bufs=2))
        combp = ctx.enter_context(tc.tile_pool(name="comb", bufs=3))
        accp = ctx.enter_context(tc.tile_pool(name="acc", bufs=3))
        accrp = ctx.enter_context(tc.tile_pool(name="accr", bufs=2))
        tmpp = ctx.enter_context(tc.tile_pool(name="tmp", bufs=3))
        psmain = ctx.enter_context(tc.tile_pool(name="psmain", bufs=2, space="PSUM"))
        psvdot = ctx.enter_context(tc.tile_pool(name="psvdot", bufs=2, space="PSUM"))
        psw1q = ctx.enter_context(tc.tile_pool(name="psw1q", bufs=2, space="PSUM"))

        # --- staged input DMAs ---------------------------------------------
        kt0 = keysp.tile([128, KB, ST], f32r)
        nc.sync.dma_start(kt0[:], keysT_r[:, :, 0, 0:ST])

        q_sb = singles.tile([128, KB, BL], f32r)
        nc.scalar.dma_start(q_sb[:], qT)
        v_sb = singles.tile([128, JB], f32)
        nc.scalar.dma_start(v_sb[:], vcol)
        o_one = singles.tile([128, BL * BL], f32r)
        nc.scalar.dma_start(o_one[:], onesz)
        ma_sb = singles.tile([BL, S], f32)
        nc.scalar.dma_start(ma_sb[:], maskadd)

        w1_sb = singles.tile([128, KB, A], f32r)
        w2_sb = singles.tile([128, KB, A], f32r)
        # j=0 columns of W1 and W2 first (on the scalar queue, parallel with
        # the first keys tile on sync), then w2/w1 column pairs per j in the
        # order the main loop consumes them
        nc.scalar.dma_start(w1_sb[:, :, 0:128], w1t[:, :, 0:128])
        nc.scalar.dma_start(w2_sb[:, :, 0:128], w2t[:, :, 0:128])
        for j in range(1, JB):
            sl = slice(j * 128, (j + 1) * 128)
            nc.scalar.dma_start(w2_sb[:, :, sl], w2t[:, :, sl])
            nc.scalar.dma_start(w1_sb[:, :, sl], w1t[:, :, sl])

        scores = singles.tile([BL, S], f32)
        mx4 = singles.tile([BL, NST], f32)
        w1q = singles.tile([128, JB, BL], f32)

        # w1q[a, b] = sum_d W1[a, d] q[b, d], a on partitions
        for j in range(JB):
            wq_ps = psw1q.tile([128, BL], f32)
            for k in range(KB):
                nc.tensor.matmul(
                    wq_ps[:],
                    lhsT=w1_sb[:, k, j * 128 : (j + 1) * 128],
                    rhs=q_sb[:, k, :],
                    start=(k == 0),
                    stop=(k == KB - 1),
                )
            nc.scalar.copy(w1q[:, j, :], wq_ps[:])

        # --- main loop ------------------------------------------------------
        for st in range(NST):
            sc_ps = psvdot.tile([BL, ST], f32)
            for b in range(BL):
                if st == 0 and b == 0:
                    kt = kt0
                else:
                    kt = keysp.tile([128, KB, ST], f32r)
                    nc.sync.dma_start(
                        kt[:], keysT_r[:, :, b, st * ST : (st + 1) * ST]
                    )
                acc = accp.tile([128, ST], f32)
                accr = accrp.tile([128, ST], f32r)
                for j in range(JB):
                    ps = psmain.tile([128, ST], f32)
                    for k in range(KB):
                        nc.tensor.matmul(
                            ps[:],
                            lhsT=w2_sb[:, k, j * 128 : (j + 1) * 128],
                            rhs=kt[:, k, :],
                            start=(k == 0),
                            stop=(k == KB - 1),
                        )
                    comb = combp.tile([128, ST], f32)
                    nc.scalar.activation(
                        comb[:], ps[:], Tanh, bias=w1q[:, j, b : b + 1]
                    )
                    # acc += v_j * comb on VectorE
                    if j == 0:
                        nc.vector.tensor_scalar_mul(acc[:], comb[:], v_sb[:, 0:1])
                    else:
                        tmp = tmpp.tile([128, ST], f32)
                        nc.vector.tensor_scalar_mul(
                            tmp[:], comb[:], v_sb[:, j : j + 1]
                        )
                        if j == JB - 1:
                            nc.vector.tensor_add(accr[:], acc[:], tmp[:])
                        else:
                            nc.vector.tensor_add(acc[:], acc[:], tmp[:])
                # partition-reduce acc into row b of sc_ps
                nc.tensor.matmul(
                    sc_ps[:],
                    lhsT=o_one[:, b * BL : (b + 1) * BL],
                    rhs=accr[:],
                    start=(b == 0),
                    stop=(b == BL - 1),
                )
            nc.vector.tensor_add(
                scores[:, st * ST : (st + 1) * ST],
                sc_ps[:, :],
                ma_sb[:, st * ST : (st + 1) * ST],
            )
            nc.vector.tensor_reduce(
                mx4[:, st : st + 1],
                scores[:, st * ST : (st + 1) * ST],
                axis=mybir.AxisListType.X,
                op=mybir.AluOpType.max,
            )

        # --- masked softmax over S for the 4 batch rows ---------------------
        nmx = singles.tile([BL, 1], f32)
        nc.vector.tensor_reduce(
            nmx[:],
            mx4[:],
            axis=mybir.AxisListType.X,
            op=mybir.AluOpType.max,
            negate=True,
        )
        e_sb = singles.tile([BL, S], f32)
        sm = singles.tile([BL, 1], f32)
        nc.scalar.activation(e_sb[:], scores[:], Exp, bias=nmx[:, 0:1], accum_out=sm[:])
        rs = singles.tile([BL, 1], f32)
        nc.vector.reciprocal(rs[:], sm[:])
        o_sb = singles.tile([BL, S], f32)
        nc.vector.tensor_scalar_mul(o_sb[:], e_sb[:], rs[:, 0:1])
        nc.sync.dma_start(out, o_sb[:])

    nc.compile()
    return nc


def _get_nc():
    if "nc" not in _cache:
        _cache["nc"] = _build_nc()
    return _cache["nc"]


def _make_in_maps(query, keys, mask, W1, W2, v):
    query = np.asarray(query, dtype=np.float32)
    keys = np.asarray(keys, dtype=np.float32)
    mask = np.asarray(mask)
    W1 = np.asarray(W1, dtype=np.float32)
    W2 = np.asarray(W2, dtype=np.float32)
    v = np.asarray(v, dtype=np.float32)

    # replicated weight layouts
    w2t = np.ascontiguousarray(W2.T.reshape(KB, 128, A).transpose(1, 0, 2))
    w1t = np.ascontiguousarray(W1.T.reshape(KB, 128, A).transpose(1, 0, 2))
    vcol = np.ascontiguousarray(v.reshape(JB, 128).T)  # [p, j]
    onesz = np.zeros((128, BL, BL), dtype=np.float32)
    for b in range(BL):
        onesz[:, b, b] = 1.0
    onesz = np.ascontiguousarray(onesz.reshape(128, BL * BL))

    in_maps = []
    for c in range(NCORES):
        sl = slice(c * BL, (c + 1) * BL)
        keysT_c = np.ascontiguousarray(keys[sl].transpose(2, 0, 1))  # (D, BL, S)
        qT_c = np.ascontiguousarray(
            query[sl].T.reshape(KB, 128, BL).transpose(1, 0, 2)
        )  # (128, KB, BL)
        maskadd_c = np.where(mask[sl], 0.0, -MASK_NEG).astype(np.float32)
        in_maps.append(
            {
                "keysT": keysT_c,
                "w2t": w2t,
                "w1t": w1t,
                "qT": qT_c,
                "vcol": vcol,
                "onesz": onesz,
                "maskadd": maskadd_c,
            }
        )
    return in_maps


def kernel(query, keys, mask, W1, W2, v):
    from concourse.bass_utils import run_bass_kernel_spmd

    nc = _get_nc()
    in_maps = _make_in_maps(query, keys, mask, W1, W2, v)
    res = run_bass_kernel_spmd(nc, in_maps, core_ids=list(range(NCORES)))
    _cache["last_results"] = res
    out = np.concatenate([res.results[i]["out"] for i in range(NCORES)], axis=0)
    return out.astype(np.float32)


# revision 12
# speedup vs baseline: 1.0517x; 1.0517x over previous
"""Additive (Bahdanau) attention kernel for one TRN2 chip (8 NeuronCores).

Computes, for query (B,D), keys (B,S,D), mask (B,S), W1 (A,D), W2 (A,D), v (A,):
    scores[b,s] = v . tanh(W1 @ query[b] + W2 @ keys[b,s])
    out = softmax(scores - 1e30 * ~mask, axis=-1)

Sharding: data-parallel over batch B across the 8 cores (4 batches/core);
W1/W2/v replicated. No collectives needed; per-core outputs are concatenated
on the host.

Per-core device kernel (main matmuls in float32r at full PE rate):
  - w1q[a,b]    = W1 @ q_b              (tiny matmul, a on partitions)
  - per (s-tile of 512, b), per a-block j of 128:
        psum[a,s] += W2T_blk.T @ keysT_tile    (8 k-blocks, PE)
        comb = tanh(psum + w1q_j[:,b])         (ScalarE, per-partition bias)
        acc  += v_j * comb                     (VectorE mul+add chain)
    last add writes acc in f32r; a one-hot ones matmul per (s-tile, b)
    partition-reduces acc into row b of a shared [4, 512] psum tile
  - scores[:, s-tile] = sc_psum + maskadd      (additive -1e30 mask)
  - running row-max per s-tile; softmax tail: exp(+bias, accum sum) ->
    recip -> scale

Weights are stored per a-block (contiguous 512 KB DRAM blocks, one SBUF tile
each) so Tile's per-tile dependency tracking lets the j-th matmul group start
as soon as its own block has landed. Keys stream on the sync DMA queue,
weights on the scalar queue, ordered to stay ahead of PE consumption; a short
burst of junk matmuls warms the PE HAM clock gate during the initial DMA wait.
"""

import numpy as np

B, S, D, A = 32, 2048, 1024, 1024
NCORES = 8
BL = B // NCORES  # 4 batches per core
ST = 512          # s-tile width
NST = S // ST     # 4 s-tiles per batch
KB = D // 128     # 8 contraction blocks
JB = A // 128     # 8 attn-dim blocks
MASK_NEG = 1e30

_cache = {}


def _build_nc():
    from contextlib import ExitStack

    import concourse.tile as tile
    from concourse import bacc, mybir

    f32 = mybir.dt.float32
    f32r = mybir.dt.float32r
    Tanh = mybir.ActivationFunctionType.Tanh
    Exp = mybir.ActivationFunctionType.Exp

    nc = bacc.Bacc(
        "TRN2",
        target_bir_lowering=False,
        debug=False,
        enable_asserts=False,
        num_devices=NCORES,
    )

    keysT = nc.dram_tensor("keysT", [D, BL, S], f32r, kind="ExternalInput").ap()
    # per-a-block weight blocks: [j, p, k*128+ai] = W[j*128+ai, k*128+p]
    w2t = nc.dram_tensor("w2t", [JB, 128, KB * 128], f32r, kind="ExternalInput").ap()
    w1t = nc.dram_tensor("w1t", [JB, 128, KB * 128], f32r, kind="ExternalInput").ap()
    qT = nc.dram_tensor("qT", [128, KB, BL], f32r, kind="ExternalInput").ap()
    # vcol[p, j] = v[j*128+p] — per-partition scalar for the DVE multiply
    vcol = nc.dram_tensor("vcol", [128, JB], f32, kind="ExternalInput").ap()
    # onesz[p, b, c] = 1 if b == c else 0 — one-hot ones column per batch so
    # each batch's partition-reduce lands in its own psum row
    onesz = nc.dram_tensor("onesz", [128, BL * BL], f32r, kind="ExternalInput").ap()
    maskadd = nc.dram_tensor("maskadd", [BL, S], f32, kind="ExternalInput").ap()
    out = nc.dram_tensor("out", [BL, S], f32, kind="ExternalOutput").ap()

    keysT_r = keysT.rearrange("(k p) b s -> p k b s", p=128)

    with tile.TileContext(nc) as tc, ExitStack() as ctx:
        singles = ctx.enter_context(tc.tile_pool(name="singles", bufs=1))
        keysp = ctx.enter_context(tc.tile_pool(name="keys", bufs=2))
        combp = ctx.enter_context(tc.tile_pool(name="comb", bufs=3))
        accp = ctx.enter_context(tc.tile_pool(name="acc", bufs=3))
        accrp = ctx.enter_context(tc.tile_pool(name="accr", bufs=2))
        tmpp = ctx.enter_context(tc.tile_pool(name="tmp", bufs=3))
        psmain = ctx.enter_context(tc.tile_pool(name="psmain", bufs=2, space="PSUM"))
        psvdot = ctx.enter_context(tc.tile_pool(name="psvdot", bufs=2, space="PSUM"))
        psw1q = ctx.enter_context(tc.tile_pool(name="psw1q", bufs=2, space="PSUM"))

        # --- staged input DMAs ---------------------------------------------
        # scalar HWDGE queue: small inputs, W1 j=0, W2 j=0, then W1 j=1..7
        q_sb = singles.tile([128, KB, BL], f32r)
        nc.scalar.dma_start(q_sb[:], qT)
        v_sb = singles.tile([128, JB], f32)
        nc.scalar.dma_start(v_sb[:], vcol)
        o_one = singles.tile([128, BL * BL], f32r)
        nc.scalar.dma_start(o_one[:], onesz)
        ma_sb = singles.tile([BL, S], f32)
        nc.scalar.dma_start(ma_sb[:], maskadd)

        w1_sbj = [singles.tile([128, KB * 128], f32r, name=f"w1_sb{j}") for j in range(JB)]
        w2_sbj = [singles.tile([128, KB * 128], f32r, name=f"w2_sb{j}") for j in range(JB)]
        nc.scalar.dma_start(w1_sbj[0][:], w1t[0])
        nc.scalar.dma_start(w2_sbj[0][:], w2t[0])
        for j in range(1, JB):
            nc.scalar.dma_start(w1_sbj[j][:], w1t[j])

        # sync HWDGE queue: first keys tile, W2 j=1..7, then the keys stream
        kt0 = keysp.tile([128, KB, ST], f32r)
        nc.sync.dma_start(kt0[:], keysT_r[:, :, 0, 0:ST])
        for j in range(1, JB):
            nc.sync.dma_start(w2_sbj[j][:], w2t[j])

        scores = singles.tile([BL, S], f32)
        mx4 = singles.tile([BL, NST], f32)
        w1qj = [singles.tile([128, BL], f32, name=f"w1q{j}") for j in range(JB)]

        # HAM warmup: junk matmuls on the first-arriving input keep the PE
        # busy through the clock-gate window while the real data streams in
        warm_ps = psw1q.tile([BL, BL], f32)
        for w in range(48):
            nc.tensor.matmul(
                warm_ps[:],
                lhsT=q_sb[:, w % KB, :],
                rhs=q_sb[:, (w + 1) % KB, :],
                start=(w == 0),
                stop=(w == 47),
            )

        # w1q_j[a, b] = sum_d W1[a, d] q[b, d], a on partitions
        for j in range(JB):
            wq_ps = psw1q.tile([128, BL], f32)
            for k in range(KB):
                nc.tensor.matmul(
                    wq_ps[:],
                    lhsT=w1_sbj[j][:, k * 128 : (k + 1) * 128],
                    rhs=q_sb[:, k, :],
                    start=(k == 0),
                    stop=(k == KB - 1),
                )
            nc.scalar.copy(w1qj[j][:], wq_ps[:])

        # --- main loop ------------------------------------------------------
        for st in range(NST):
            sc_ps = psvdot.tile([BL, ST], f32)
            for b in range(BL):
                if st == 0 and b == 0:
                    kt = kt0
                else:
                    kt = keysp.tile([128, KB, ST], f32r)
                    nc.sync.dma_start(
                        kt[:], keysT_r[:, :, b, st * ST : (st + 1) * ST]
                    )
                acc = accp.tile([128, ST], f32)
                accr = accrp.tile([128, ST], f32r)
                for j in range(JB):
                    ps = psmain.tile([128, ST], f32)
                    for k in range(KB):
                        nc.tensor.matmul(
                            ps[:],
                            lhsT=w2_sbj[j][:, k * 128 : (k + 1) * 128],
                            rhs=kt[:, k, :],
                            start=(k == 0),
                            stop=(k == KB - 1),
                        )
                    comb = combp.tile([128, ST], f32)
                    nc.scalar.activation(
                        comb[:], ps[:], Tanh, bias=w1qj[j][:, b : b + 1]
                    )
                    # acc += v_j * comb on VectorE
                    if j == 0:
                        nc.vector.tensor_scalar_mul(acc[:], comb[:], v_sb[:, 0:1])
                    else:
                        tmp = tmpp.tile([128, ST], f32)
                        nc.vector.tensor_scalar_mul(
                            tmp[:], comb[:], v_sb[:, j : j + 1]
                        )
                        if j == JB - 1:
                            nc.vector.tensor_add(accr[:], acc[:], tmp[:])
                        else:
                            nc.vector.tensor_add(acc[:], acc[:], tmp[:])
                # partition-reduce acc into row b of sc_ps
                nc.tensor.matmul(
                    sc_ps[:],
                    lhsT=o_one[:, b * BL : (b + 1) * BL],
                    rhs=accr[:],
                    start=(b == 0),
                    stop=(b == BL - 1),
                )
            nc.vector.tensor_add(
                scores[:, st * ST : (st + 1) * ST],
                sc_ps[:, :],
                ma_sb[:, st * ST : (st + 1) * ST],
            )
            nc.vector.tensor_reduce(
                mx4[:, st : st + 1],
                scores[:, st * ST : (st + 1) * ST],
                axis=mybir.AxisListType.X,
                op=mybir.AluOpType.max,
            )

        # --- masked softmax over S for the 4 batch rows ---------------------
        nmx = singles.tile([BL, 1], f32)
        nc.vector.tensor_reduce(
            nmx[:],
            mx4[:],
            axis=mybir.AxisListType.X,
            op=mybir.AluOpType.max,
            negate=True,
        )
        e_sb = singles.tile([BL, S], f32)
        sm = singles.tile([BL, 1], f32)
        nc.scalar.activation(e_sb[:], scores[:], Exp, bias=nmx[:, 0:1], accum_out=sm[:])
        rs = singles.tile([BL, 1], f32)
        nc.vector.reciprocal(rs[:], sm[:])
        o_sb = singles.tile([BL, S], f32)
        nc.vector.tensor_scalar_mul(o_sb[:], e_sb[:], rs[:, 0:1])
        nc.sync.dma_start(out, o_sb[:])

    nc.compile()
    return nc


def _get_nc():
    if "nc" not in _cache:
        _cache["nc"] = _build_nc()
    return _cache["nc"]


def _weight_blocks(W):
    # [j, p, k*128+ai] = W[j*128+ai, k*128+p]
    return np.ascontiguousarray(
        W.reshape(JB, 128, KB, 128).transpose(0, 3, 2, 1).reshape(JB, 128, KB * 128)
    )


def _make_in_maps(query, keys, mask, W1, W2, v):
    query = np.asarray(query, dtype=np.float32)
    keys = np.asarray(keys, dtype=np.float32)
    mask = np.asarray(mask)
    W1 = np.asarray(W1, dtype=np.float32)
    W2 = np.asarray(W2, dtype=np.float32)
    v = np.asarray(v, dtype=np.float32)

    w2t = _weight_blocks(W2)
    w1t = _weight_blocks(W1)
    vcol = np.ascontiguousarray(v.reshape(JB, 128).T)  # [p, j]
    onesz = np.zeros((128, BL, BL), dtype=np.float32)
    for b in range(BL):
        onesz[:, b, b] = 1.0
    onesz = np.ascontiguousarray(onesz.reshape(128, BL * BL))

    in_maps = []
    for c in range(NCORES):
        sl = slice(c * BL, (c + 1) * BL)
        keysT_c = np.ascontiguousarray(keys[sl].transpose(2, 0, 1))  # (D, BL, S)
        qT_c = np.ascontiguousarray(
            query[sl].T.reshape(KB, 128, BL).transpose(1, 0, 2)
        )  # (128, KB, BL)
        maskadd_c = np.where(mask[sl], 0.0, -MASK_NEG).astype(np.float32)
        in_maps.append(
            {
                "keysT": keysT_c,
                "w2t": w2t,
                "w1t": w1t,
                "qT": qT_c,
                "vcol": vcol,
                "onesz": onesz,
                "maskadd": maskadd_c,
            }
        )
    return in_maps


def kernel(query, keys, mask, W1, W2, v):
    from concourse.bass_utils import run_bass_kernel_spmd

    nc = _get_nc()
    in_maps = _make_in_maps(query, keys, mask, W1, W2, v)
    res = run_bass_kernel_spmd(nc, in_maps, core_ids=list(range(NCORES)))
    _cache["last_results"] = res
    out = np.concatenate([res.results[i]["out"] for i in range(NCORES)], axis=0)
    return out.astype(np.float32)


# revision 16
# speedup vs baseline: 1.0927x; 1.0390x over previous
"""Additive (Bahdanau) attention kernel for one TRN2 chip (8 NeuronCores).

Computes, for query (B,D), keys (B,S,D), mask (B,S), W1 (A,D), W2 (A,D), v (A,):
    scores[b,s] = v . tanh(W1 @ query[b] + W2 @ keys[b,s])
    out = softmax(scores - 1e30 * ~mask, axis=-1)

Sharding: data-parallel over batch B across the 8 cores (4 batches/core);
W1/W2/v replicated. No collectives needed; per-core outputs are concatenated
on the host.

Per-core device kernel (main matmuls in float32r at full PE rate):
  - w1q[a,b]    = W1 @ q_b              (tiny matmul, a on partitions)
  - per (s-tile of 512, b), per a-block j of 128:
        psum[a,s] += W2T_blk.T @ keysT_tile    (8 k-blocks, PE)
        comb = tanh(psum + w1q_j[:,b])         (ScalarE, per-partition bias)
        acc  += v_j * comb                     (VectorE mul+add chain)
    last add writes acc in f32r; a one-hot ones matmul per (s-tile, b)
    partition-reduces acc into row b of a shared [4, 512] psum tile
  - scores[:, s-tile] = sc_psum + maskadd      (additive -1e30 mask)
  - running row-max per s-tile; softmax tail: exp(+bias, accum sum) ->
    recip -> scale

Weights are stored per a-block (contiguous 512 KB DRAM blocks, one SBUF tile
each) so Tile's per-tile dependency tracking lets the j-th matmul group start
as soon as its own block has landed. Keys stream on the sync DMA queue,
weights on the scalar queue, ordered to stay ahead of PE consumption; a short
burst of junk matmuls warms the PE HAM clock gate during the initial DMA wait.
"""

import numpy as np

B, S, D, A = 32, 2048, 1024, 1024
NCORES = 8
BL = B // NCORES  # 4 batches per core
ST = 512          # s-tile width
NST = S // ST     # 4 s-tiles per batch
KB = D // 128     # 8 contraction blocks
JB = A // 128     # 8 attn-dim blocks
MASK_NEG = 1e30

_cache = {}


def _build_nc():
    from contextlib import ExitStack

    import concourse.tile as tile
    from concourse import bacc, mybir

    f32 = mybir.dt.float32
    f32r = mybir.dt.float32r
    bf16 = mybir.dt.bfloat16
    Tanh = mybir.ActivationFunctionType.Tanh
    Exp = mybir.ActivationFunctionType.Exp

    nc = bacc.Bacc(
        "TRN2",
        target_bir_lowering=False,
        debug=False,
        enable_asserts=False,
        num_devices=NCORES,
    )

    keysT = nc.dram_tensor("keysT", [D, BL, S], f32r, kind="ExternalInput").ap()
    # per-a-block weight blocks: [j, p, k*128+ai] = W[j*128+ai, k*128+p]
    w2t = nc.dram_tensor("w2t", [JB, 128, KB * 128], f32r, kind="ExternalInput").ap()
    w1t = nc.dram_tensor("w1t", [JB, 128, KB * 128], bf16, kind="ExternalInput").ap()
    qT = nc.dram_tensor("qT", [128, KB, BL], f32r, kind="ExternalInput").ap()
    qTb = nc.dram_tensor("qTb", [128, KB, BL], bf16, kind="ExternalInput").ap()
    # vcol[p, j] = v[j*128+p] — per-partition scalar for the DVE multiply
    vcol = nc.dram_tensor("vcol", [128, JB], f32, kind="ExternalInput").ap()
    # onesz[p, b, c] = 1 if b == c else 0 — one-hot ones column per batch so
    # each batch's partition-reduce lands in its own psum row
    onesz = nc.dram_tensor("onesz", [128, BL * BL], f32r, kind="ExternalInput").ap()
    maskadd = nc.dram_tensor("maskadd", [BL, S], f32, kind="ExternalInput").ap()
    out = nc.dram_tensor("out", [BL, S], f32, kind="ExternalOutput").ap()

    keysT_r = keysT.rearrange("(k p) b s -> p k b s", p=128)

    with tile.TileContext(nc) as tc, ExitStack() as ctx:
        singles = ctx.enter_context(tc.tile_pool(name="singles", bufs=1))
        keysp = ctx.enter_context(tc.tile_pool(name="keys", bufs=2))
        combp = ctx.enter_context(tc.tile_pool(name="comb", bufs=3))
        accp = ctx.enter_context(tc.tile_pool(name="acc", bufs=3))
        accrp = ctx.enter_context(tc.tile_pool(name="accr", bufs=2))
        tmpp = ctx.enter_context(tc.tile_pool(name="tmp", bufs=3))
        psmain = ctx.enter_context(tc.tile_pool(name="psmain", bufs=2, space="PSUM"))
        psvdot = ctx.enter_context(tc.tile_pool(name="psvdot", bufs=2, space="PSUM"))
        psw1q = ctx.enter_context(tc.tile_pool(name="psw1q", bufs=2, space="PSUM"))

        # --- staged input DMAs ---------------------------------------------
        # sync HWDGE queue (fast): q for warmup, W2 j=0, first keys tile,
        # then W2 j=1..7 ahead of the keys stream
        q_sb = singles.tile([128, KB, BL], f32r)
        nc.sync.dma_start(q_sb[:], qT)
        w2_sbj = [singles.tile([128, KB * 128], f32r, name=f"w2_sb{j}") for j in range(JB)]
        nc.sync.dma_start(w2_sbj[0][:], w2t[0])
        kt0 = keysp.tile([128, KB, ST], f32r)
        nc.sync.dma_start(kt0[:], keysT_r[:, :, 0, 0:ST])
        for j in range(1, JB):
            nc.sync.dma_start(w2_sbj[j][:], w2t[j])

        # scalar HWDGE queue (slower, starts later): everything the tanh-bias
        # path needs — W1 in bf16 halves its bytes so w1q_j7 beats its deadline
        v_sb = singles.tile([128, JB], f32)
        nc.scalar.dma_start(v_sb[:], vcol)
        o_one = singles.tile([128, BL * BL], f32r)
        nc.scalar.dma_start(o_one[:], onesz)
        qb_sb = singles.tile([128, KB, BL], bf16)
        nc.scalar.dma_start(qb_sb[:], qTb)
        w1_sbj = [singles.tile([128, KB * 128], bf16, name=f"w1_sb{j}") for j in range(JB)]
        for j in range(JB):
            nc.scalar.dma_start(w1_sbj[j][:], w1t[j])
        ma_sb = singles.tile([BL, S], f32)
        nc.scalar.dma_start(ma_sb[:], maskadd)

        scores = singles.tile([BL, S], f32)
        mx4 = singles.tile([BL, NST], f32)
        w1qj = [singles.tile([128, BL], f32, name=f"w1q{j}") for j in range(JB)]

        # HAM warmup: junk matmuls on the first-arriving input keep the PE
        # busy through the clock-gate window while the real data streams in
        warm_ps = psw1q.tile([BL, KB * BL], f32)
        q_flat = q_sb[:].rearrange("p k b -> p (k b)")
        for w in range(96):
            nc.tensor.matmul(
                warm_ps[:],
                lhsT=q_sb[:, w % KB, :],
                rhs=q_flat,
                start=(w == 0),
                stop=(w == 95),
            )

        # w1q_j[a, b] = sum_d W1[a, d] q[b, d], a on partitions
        for j in range(JB):
            wq_ps = psw1q.tile([128, BL], f32)
            for k in range(KB):
                nc.tensor.matmul(
                    wq_ps[:],
                    lhsT=w1_sbj[j][:, k * 128 : (k + 1) * 128],
                    rhs=qb_sb[:, k, :],
                    start=(k == 0),
                    stop=(k == KB - 1),
                )
            nc.scalar.copy(w1qj[j][:], wq_ps[:])

        # --- main loop ------------------------------------------------------
        # the one-hot partition-reduce matmul for (st, b) is emitted two main
        # j-groups later so the in-order PE never waits on the tanh+DVE chain
        groups_done = 0
        ones_queue = []  # (sc_ps, accr, b, st, group_when_ready)

        def flush_ones(min_age):
            while ones_queue and groups_done - ones_queue[0][4] >= min_age:
                sc_ps_q, accr_q, b_q, st_q, _ = ones_queue.pop(0)
                nc.tensor.matmul(
                    sc_ps_q[:],
                    lhsT=o_one[:, b_q * BL : (b_q + 1) * BL],
                    rhs=accr_q[:],
                    start=(b_q == 0),
                    stop=(b_q == BL - 1),
                )
                if b_q == BL - 1:
                    nc.vector.tensor_add(
                        scores[:, st_q * ST : (st_q + 1) * ST],
                        sc_ps_q[:, :],
                        ma_sb[:, st_q * ST : (st_q + 1) * ST],
                    )
                    nc.vector.tensor_reduce(
                        mx4[:, st_q : st_q + 1],
                        scores[:, st_q * ST : (st_q + 1) * ST],
                        axis=mybir.AxisListType.X,
                        op=mybir.AluOpType.max,
                    )

        sc_tiles = [psvdot.tile([BL, ST], f32, name=f"sc_ps{st}", tag="sc_ps") for st in range(NST)]
        for st in range(NST):
            sc_ps = sc_tiles[st]
            for b in range(BL):
                if st == 0 and b == 0:
                    kt = kt0
                else:
                    kt = keysp.tile([128, KB, ST], f32r)
                    nc.sync.dma_start(
                        kt[:], keysT_r[:, :, b, st * ST : (st + 1) * ST]
                    )
                acc = accp.tile([128, ST], f32)
                accr = accrp.tile([128, ST], f32r)
                for j in range(JB):
                    ps = psmain.tile([128, ST], f32)
                    for k in range(KB):
                        nc.tensor.matmul(
                            ps[:],
                            lhsT=w2_sbj[j][:, k * 128 : (k + 1) * 128],
                            rhs=kt[:, k, :],
                            start=(k == 0),
                            stop=(k == KB - 1),
                        )
                    groups_done += 1
                    flush_ones(2)
                    comb = combp.tile([128, ST], f32)
                    nc.scalar.activation(
                        comb[:], ps[:], Tanh, bias=w1qj[j][:, b : b + 1]
                    )
                    # acc += v_j * comb on VectorE
                    if j == 0:
                        nc.vector.tensor_scalar_mul(acc[:], comb[:], v_sb[:, 0:1])
                    else:
                        tmp = tmpp.tile([128, ST], f32)
                        nc.vector.tensor_scalar_mul(
                            tmp[:], comb[:], v_sb[:, j : j + 1]
                        )
                        if j == JB - 1:
                            nc.vector.tensor_add(accr[:], acc[:], tmp[:])
                        else:
                            nc.vector.tensor_add(acc[:], acc[:], tmp[:])
                ones_queue.append((sc_ps, accr, b, st, groups_done))
        flush_ones(0)

        # --- masked softmax over S for the 4 batch rows ---------------------
        nmx = singles.tile([BL, 1], f32)
        nc.vector.tensor_reduce(
            nmx[:],
            mx4[:],
            axis=mybir.AxisListType.X,
            op=mybir.AluOpType.max,
            negate=True,
        )
        e_sb = singles.tile([BL, S], f32)
        sm = singles.tile([BL, 1], f32)
        nc.scalar.activation(e_sb[:], scores[:], Exp, bias=nmx[:, 0:1], accum_out=sm[:])
        rs = singles.tile([BL, 1], f32)
        nc.vector.reciprocal(rs[:], sm[:])
        o_sb = singles.tile([BL, S], f32)
        nc.vector.tensor_scalar_mul(o_sb[:], e_sb[:], rs[:, 0:1])
        nc.sync.dma_start(out, o_sb[:])

    nc.compile()
    return nc


def _get_nc():
    if "nc" not in _cache:
        _cache["nc"] = _build_nc()
    return _cache["nc"]


def _weight_blocks(W):
    # [j, p, k*128+ai] = W[j*128+ai, k*128+p]
    return np.ascontiguousarray(
        W.reshape(JB, 128, KB, 128).transpose(0, 3, 2, 1).reshape(JB, 128, KB * 128)
    )


def _make_in_maps(query, keys, mask, W1, W2, v):
    query = np.asarray(query, dtype=np.float32)
    keys = np.asarray(keys, dtype=np.float32)
    mask = np.asarray(mask)
    W1 = np.asarray(W1, dtype=np.float32)
    W2 = np.asarray(W2, dtype=np.float32)
    v = np.asarray(v, dtype=np.float32)

    import ml_dtypes

    w2t = _weight_blocks(W2)
    w1t = _weight_blocks(W1).astype(ml_dtypes.bfloat16)
    vcol = np.ascontiguousarray(v.reshape(JB, 128).T)  # [p, j]
    onesz = np.zeros((128, BL, BL), dtype=np.float32)
    for b in range(BL):
        onesz[:, b, b] = 1.0
    onesz = np.ascontiguousarray(onesz.reshape(128, BL * BL))

    in_maps = []
    for c in range(NCORES):
        sl = slice(c * BL, (c + 1) * BL)
        keysT_c = np.ascontiguousarray(keys[sl].transpose(2, 0, 1))  # (D, BL, S)
        qT_c = np.ascontiguousarray(
            query[sl].T.reshape(KB, 128, BL).transpose(1, 0, 2)
        )  # (128, KB, BL)
        qTb_c = qT_c.astype(ml_dtypes.bfloat16)
        maskadd_c = np.where(mask[sl], 0.0, -MASK_NEG).astype(np.float32)
        in_maps.append(
            {
                "keysT": keysT_c,
                "w2t": w2t,
                "w1t": w1t,
                "qT": qT_c,
                "qTb": qTb_c,
                "vcol": vcol,
                "onesz": onesz,
                "maskadd": maskadd_c,
            }
        )
    return in_maps


def kernel(query, keys, mask, W1, W2, v):
    from concourse.bass_utils import run_bass_kernel_spmd

    nc = _get_nc()
    in_maps = _make_in_maps(query, keys, mask, W1, W2, v)
    res = run_bass_kernel_spmd(nc, in_maps, core_ids=list(range(NCORES)))
    _cache["last_results"] = res
    out = np.concatenate([res.results[i]["out"] for i in range(NCORES)], axis=0)
    return out.astype(np.float32)
